# revision 15
# baseline (speedup 1.0000x reference)
"""MultiHead HGNN attention (B=2, S=4096, D=256, H=4) on 8 TRN2 NeuronCores.

Sharding: query rows split 8 ways (512 rows/core); every core computes all
batches/heads for its query block. Scores are built k-major (scores^T); G^T is
pre-transposed on the HOST (free) and DMA'd straight into SBUF as fp16.
Softmax denominators ride as ones-columns in the V operand; probs are fp16.

The mul-by-G + exp elementwise stage over 16.7M scores/core is split across
THREE engines per key chunk (pattern per 32-kc pair, tuned so DVE/ACT/Pool all
land ~109us):
  path a (x21): fused custom DVE op  i16 = max(s*g*A + B, 0)  writing the
          int16 bit pattern of fp16(e^(s*g)) (Schraudolph); PSUM-sourced.
  path d (x10): ACT drains scores PSUM->fp16, Pool (GpSimd) does the g-mul
          as a plain tensor_tensor mult (the only elementwise opcode walrus
          accepts on Pool; Pool also has no PSUM port, hence the drain), ACT
          does the relu(x*A+B)->i16 Schraudolph tail.
The softmax normalization divides out Schraudolph's common-mode error.

Startup: DMA order is wk, wq, xqt, xt[b0], wv, gt[0:2], gt[2:8], gt[8:32],
xt[b1] so the Q/K builds gate on ~3MB not 10MB. Only kt(0,0)+q+va(0,sg0..5)
build before the main loop; the other K/V builds are injected at fixed kc
positions inside earlier pairs' loops (they borrow scores-PSUM slots briefly
instead of serializing 48 rounds through 3 slots up front).

Pair-end softmax denominators: 1/den is broadcast across 64 partitions with a
K=1 fp32 matmul into PSUM (ones^T @ rec) instead of the previous ~3.5us SWDGE
DMA broadcast that stalled DVE at every pair boundary.
"""

import contextlib
import ctypes
import sys
import types

import numpy as np

sys.path.insert(0, "/opt/trn_rl_repo")

SCHRA_A = 1024.0 / float(np.log(2.0))   # 1477.3199 = 2^10 * log2(e)
SCHRA_B = 15360.0 - 100.0               # fp16 exponent bias<<10, sigma tuned on sim
# per-kc elementwise path pattern over the 32 key chunks of each (b,hp) pair:
# 'd' (ACT drain + Pool mul + ACT relu) every 3rd chunk, 'a' (fused DVE) rest
PATTERN32 = "".join("d" if (i % 3 == 1 and i < 29) else "a" for i in range(32))
assert len(PATTERN32) == 32


def _install_axon_hooks():
    """The agent image's antenv lacks axon_hooks; provide it so bass_utils can
    NTFF-profile under axon. Harmless when profiling is never requested."""
    if "antenv.axon_hooks" in sys.modules:
        return
    try:
        import antenv
    except ImportError:
        return
    mod = types.ModuleType("antenv.axon_hooks")
    holder = {}
    mod.set_axon_ntff_profile_hook = lambda h: holder.__setitem__("h", h)
    mod.get_axon_ntff_profile_hook = lambda: holder.get("h")
    sys.modules["antenv.axon_hooks"] = mod
    antenv.axon_hooks = mod
    try:
        lib = ctypes.CDLL("/opt/axon/libaxon_pjrt.so")
    except OSError:
        return
    if not hasattr(lib, "axon_start_nrt_profile"):
        return
    lib.axon_start_nrt_profile.argtypes = [ctypes.POINTER(ctypes.c_int64), ctypes.c_size_t]
    lib.axon_start_nrt_profile.restype = ctypes.c_int64
    lib.axon_stop_nrt_profile.argtypes = [ctypes.c_char_p]
    lib.axon_stop_nrt_profile.restype = ctypes.c_int64

    @contextlib.contextmanager
    def _hook(output_dir, device_ids):
        import jax

        jax.devices()
        if device_ids:
            ids = (ctypes.c_int64 * len(device_ids))(*device_ids)
            rc = lib.axon_start_nrt_profile(ids, len(device_ids))
        else:
            rc = lib.axon_start_nrt_profile(None, 0)
        if rc != 0:
            raise RuntimeError(f"axon_start_nrt_profile rc={rc}")
        try:
            yield
        finally:
            n = lib.axon_stop_nrt_profile(str(output_dir).encode())
            print(f"profile: {n} file(s) written to {output_dir}")

    mod.set_axon_ntff_profile_hook(_hook)


_install_axon_hooks()

B, S, D, H, HD = 2, 4096, 256, 4, 64
NCORES = 8
QR = S // NCORES          # 512 query rows per core
KC = S // 128             # 32 key chunks of 128
SCALE = 1.0 / np.sqrt(HD)

_BUILT = {}


def _register_schra_op():
    """Register the fused (Src0*Src1*C0 + C1 clamped at 0 -> int16) DVE op."""
    from concourse import dve_ops
    from concourse.dve_spec import Spec, Src0, Src1, C0, C1, Zero, maxx, lower, _has_src1
    from concourse.dve_uop import DveOpSpec

    if "schra_op" in _BUILT:
        return _BUILT["schra_op"]
    name = "SCHRA_MULADD_ANT"
    for existing in dve_ops.OPS:
        if existing.name == name:  # re-import in the same process
            _BUILT["schra_op"] = existing
            return existing
    spec = Spec(
        body=maxx(Src0 * Src1 * C0 + C1, Zero),
        reference=lambda in0, in1, s0, s1, imm2: np.maximum(
            in0.astype(np.float32) * in1.astype(np.float32) * s0 + s1, 0.0
        ).astype(np.float32),
    )
    row = dve_ops._CUSTOM_DVE_ROW_BASE + len(dve_ops.OPS)
    shas = {}
    for ver in ("v3", "v4"):
        s = DveOpSpec(name=name, opcode=row, uops=lower(spec, ver=ver), rd1_en=_has_src1(spec))
        shas[ver] = s.sha(ver)
    op = dve_ops.DveOp(name, spec, subdim=False, uops_sha=shas)
    dve_ops.OPS.append(op)
    dve_ops.CUSTOM_DVE_SPECS[name] = spec
    dve_ops._SUB_OPCODE_FOR_NAME[name] = row
    _BUILT["schra_op"] = op
    return op


def build_bass():
    if "nc" in _BUILT:
        return _BUILT["nc"]

    import concourse.tile as tile
    from concourse import bacc, mybir

    f32, bf16, f16, i16 = (
        mybir.dt.float32, mybir.dt.bfloat16, mybir.dt.float16, mybir.dt.int16,
    )
    af = mybir.ActivationFunctionType
    alu = mybir.AluOpType
    schra = _register_schra_op()

    nc = bacc.Bacc("TRN2", target_bir_lowering=False, debug=False, num_devices=NCORES)

    xt_in = nc.dram_tensor("xt", [B, 2, 128, S], bf16, kind="ExternalInput")
    xqt_in = nc.dram_tensor("xqt", [B, 2, 128, QR], bf16, kind="ExternalInput")
    g_in = nc.dram_tensor("g", [128, KC, QR], f16, kind="ExternalInput")
    wq_in = nc.dram_tensor("wq", [2, 128, 256], bf16, kind="ExternalInput")
    wk_in = nc.dram_tensor("wk", [2, 128, 256], bf16, kind="ExternalInput")
    wv_in = nc.dram_tensor("wv", [2, 128, 256], bf16, kind="ExternalInput")
    wo_in = nc.dram_tensor("wo", [H, 64, 256], bf16, kind="ExternalInput")
    bias_in = nc.dram_tensor("bias", [1, 256], bf16, kind="ExternalInput")
    out_dram = nc.dram_tensor("out", [B, QR, 256], f32, kind="ExternalOutput")

    with tile.TileContext(nc) as tc, contextlib.ExitStack() as ctx:
        cp = ctx.enter_context(tc.tile_pool(name="const", bufs=1))
        # 3 slots x 2 banks for scores / KVQ staging / out-proj / rec-broadcast
        ps_big = ctx.enter_context(tc.tile_pool(name="ps_big", bufs=3, space="PSUM"))
        ps_ct = ctx.enter_context(tc.tile_pool(name="ps_ct", bufs=1, space="PSUM"))

        # ---- weights, DMA'd in dependency order: wk/wq gate the K/Q builds
        wk_sb = cp.tile([128, 2, 256], bf16, tag="wk")
        wq_sb = cp.tile([128, 2, 256], bf16, tag="wq")
        wv_sb = cp.tile([128, 2, 256], bf16, tag="wv")
        bias_sb = cp.tile([1, 256], bf16, tag="bias")
        ones_sb = cp.tile([1, 128], bf16, tag="ones")
        ones32_sb = cp.tile([1, 64], f32, tag="ones32")
        schrab_sb = cp.tile([128, 1], f32, tag="schrab")
        nc.gpsimd.memset(schrab_sb[:], SCHRA_B)
        nc.gpsimd.memset(ones_sb[:], 1.0)
        nc.gpsimd.memset(ones32_sb[:], 1.0)
        for ic in range(2):
            nc.sync.dma_start(wk_sb[:, ic, :], wk_in[ic])
            nc.sync.dma_start(wq_sb[:, ic, :], wq_in[ic])

        # xqt before xt so the Q projection (which gates the first scores)
        # isn't queued behind 4MB of x^T
        xqt_sb = cp.tile([128, B, 2, QR], bf16, tag="xqt")
        for b in range(B):
            for ic in range(2):
                nc.sync.dma_start(xqt_sb[:, b, ic, :], xqt_in[b, ic])

        xt_sb = [[cp.tile([128, S], bf16, tag=f"xt{b}{ic}", name=f"xt{b}{ic}") for ic in range(2)] for b in range(B)]
        for ic in range(2):
            nc.sync.dma_start(xt_sb[0][ic][:], xt_in[0, ic])
        for ic in range(2):
            nc.sync.dma_start(wv_sb[:, ic, :], wv_in[ic])

        # G^T: host-pretransposed; first 2 kc land within ~10us so pair-0
        # elementwise never waits, bulk follows, then batch-1 x^T
        gt_sb = cp.tile([128, KC, QR], f16, tag="gt")
        nc.sync.dma_start(gt_sb[:, 0:2, :], g_in[:, 0:2, :])
        nc.sync.dma_start(gt_sb[:, 2:8, :], g_in[:, 2:8, :])
        for kp in range(1, 4):
            nc.sync.dma_start(gt_sb[:, kp * 8:(kp + 1) * 8, :], g_in[:, kp * 8:(kp + 1) * 8, :])
        for ic in range(2):
            nc.sync.dma_start(xt_sb[1][ic][:], xt_in[1, ic])
        wo_sb = []
        for h in range(H):
            t = cp.tile([64, 256], bf16, tag=f"wo{h}", name=f"wo{h}")
            nc.sync.dma_start(t[:], wo_in[h])
            wo_sb.append(t)
        nc.sync.dma_start(bias_sb[:], bias_in[:])

        qts = [[cp.tile([128, QR], bf16, tag=f"qt{b}{hp}", name=f"qt{b}{hp}") for hp in range(2)] for b in range(B)]

        vap = ctx.enter_context(tc.tile_pool(name="vap", bufs=2))
        stp = ctx.enter_context(tc.tile_pool(name="stp", bufs=4))
        ttp = ctx.enter_context(tc.tile_pool(name="ttp", bufs=4))
        pp = ctx.enter_context(tc.tile_pool(name="pp", bufs=5))
        otp = ctx.enter_context(tc.tile_pool(name="otp", bufs=2))
        rp = ctx.enter_context(tc.tile_pool(name="rp", bufs=2))

        ctf = [[None] * H for _ in range(B)]

        # va[keys, kc, 65h:65h+64] per batch, ones col at 65h+64 (denominators,
        # memset once up front; the per-sg copies skip those columns)
        vas = [vap.tile([128, KC, 260], f16, tag="va", name=f"va{b}") for b in range(B)]
        for b in range(B):
            nc.vector.memset(
                vas[b][:, :, :].rearrange("p k (h x) -> p k h x", h=4)[:, :, :, 64:65], 1.0
            )
        kts = [[cp.tile([128, S], bf16, tag=f"kt{b}{hp}", name=f"kt{b}{hp}") for hp in range(2)] for b in range(B)]

        def q_build(b, hp):
            aux = ps_big.tile([128, 2, 512], f32, tag="sc", name="auxq")
            for ic in range(2):
                nc.tensor.matmul(
                    aux[:, 0, :QR], wq_sb[:, ic, hp * 128:(hp + 1) * 128],
                    xqt_sb[:, b, ic, :], start=(ic == 0), stop=(ic == 1),
                )
            nc.vector.tensor_copy(qts[b][hp][:], aux[:, 0, :QR])

        def va_round(b, sg):
            va = vas[b]
            vps = ps_big.tile([128, 2, 512], f32, tag="sc", name="auxv")
            for j in range(2):
                kcj = sg * 2 + j
                for ic in range(2):
                    nc.tensor.matmul(
                        vps[:, j, 0:256],
                        xt_sb[b][ic][:, kcj * 128:(kcj + 1) * 128],
                        wv_sb[:, ic, :],
                        start=(ic == 0), stop=(ic == 1),
                    )
            nc.scalar.copy(
                va[:, sg * 2:(sg + 1) * 2, :].rearrange("p k (h x) -> p k h x", h=4)[:, :, :, 0:64],
                vps[:, 0:2, 0:256].rearrange("p j (h x) -> p j h x", h=4),
            )

        def kt_round(b, hp, sc4):
            kt = kts[b][hp]
            auxk = ps_big.tile([128, 2, 512], f32, tag="sc", name="auxk")
            for half in range(2):
                sc8 = sc4 * 2 + half
                for ic in range(2):
                    nc.tensor.matmul(
                        auxk[:, half, :], wk_sb[:, ic, hp * 128:(hp + 1) * 128],
                        xt_sb[b][ic][:, sc8 * 512:(sc8 + 1) * 512],
                        start=(ic == 0), stop=(ic == 1),
                    )
            nc.scalar.copy(
                kt[:, sc4 * 1024:(sc4 + 1) * 1024],
                auxk[:, 0:2, :].rearrange("p a b -> p (a b)"),
            )

        def out_proj(b):
            for qs in range(QR // 128):
                op = ps_big.tile([128, 2, 512], f32, tag="sc", name="auxo")
                for h in range(H):
                    nc.tensor.matmul(
                        op[:, 0, 0:256], ctf[b][h][:, qs * 128:(qs + 1) * 128],
                        wo_sb[h][:], start=(h == 0), stop=False,
                    )
                nc.tensor.matmul(op[:, 0, 0:256], ones_sb[0:1, :], bias_sb[0:1, :], start=False, stop=True)
                ot = otp.tile([128, 256], f32, tag="ot")
                nc.vector.tensor_copy(ot[:], op[:, 0, 0:256])
                nc.sync.dma_start(out_dram[b, qs * 128:(qs + 1) * 128, :], ot[:])

        # ---- minimal upfront builds; the rest is injected into pair loops
        for b in range(B):
            for hp in range(2):
                q_build(b, hp)
        for sc4 in range(4):
            kt_round(0, 0, sc4)
        for sg in range(6):
            va_round(0, sg)

        sched = {}

        def add(pidx, kc, fn):
            sched.setdefault((pidx, kc), []).append(fn)

        for i, sg in enumerate(range(6, 16)):          # va(0) tail
            add(0, 1 + 2 * i, lambda s=sg: va_round(0, s))
        for i, s4 in enumerate(range(4)):              # kt(0,1)
            add(0, 21 + 2 * i, lambda s=s4: kt_round(0, 1, s))
        for sg in range(16):                           # va(1)
            add(1, 1 + 2 * sg, lambda s=sg: va_round(1, s))
        for i, s4 in enumerate(range(4)):              # kt(1,0)
            add(1, 22 + 2 * i, lambda s=s4: kt_round(1, 0, s))
        for i, s4 in enumerate(range(4)):              # kt(1,1)
            add(2, 2 + 2 * i, lambda s=s4: kt_round(1, 1, s))
        add(2, 8, lambda: out_proj(0))

        for pidx, (b, hp) in enumerate([(0, 0), (0, 1), (1, 0), (1, 1)]):
            va = vas[b]
            qt = qts[b][hp]
            kt = kts[b][hp]

            # ---- main loop over key chunks ----
            ct0 = ps_ct.tile([65, QR], f32, tag="ct0")
            ct1 = ps_ct.tile([65, QR], f32, tag="ct1")
            for kc in range(KC):
                scp = ps_big.tile([128, 2, QR], f32, tag="sc", name="scp")
                nc.tensor.matmul(
                    scp[:, 0, :], kt[0:64, kc * 128:(kc + 1) * 128], qt[0:64, :],
                    start=True, stop=True, tile_position=(0, 0),
                )
                nc.tensor.matmul(
                    scp[:, 1, :], kt[64:128, kc * 128:(kc + 1) * 128], qt[64:128, :],
                    start=True, stop=True, tile_position=(64, 0),
                )
                pt = pp.tile([128, 2, QR], f16, tag="pt")
                path = PATTERN32[kc]
                gtb = gt_sb[:, kc:kc + 1, :].broadcast_to([128, 2, QR])
                if path == "a":
                    # fused: i16 bits of fp16(e^(s*g)) straight from PSUM
                    nc.vector._custom_dve(
                        schra,
                        out=pt[:, :, :].bitcast(i16),
                        in0=scp[:, :, :],
                        in1=gtb,
                        s0=SCHRA_A, s1=SCHRA_B,
                    )
                else:
                    sc16 = stp.tile([128, 2, QR], f16, tag="sc16")
                    nc.scalar.copy(sc16[:, :, :], scp[:, :, :])
                    tt = ttp.tile([128, 2, QR], f16, tag="tt")
                    nc.gpsimd.tensor_mul(tt[:, :, :], sc16[:, :, :], gtb)
                    # Schraudolph tail on ACT: relu(t*A+B) -> i16
                    nc.scalar.activation(
                        pt[:, :, :].rearrange("p a b -> p (a b)").bitcast(i16),
                        tt[:, :, :].rearrange("p a b -> p (a b)"),
                        af.Relu, scale=SCHRA_A, bias=schrab_sb[:, :],
                    )
                h0 = 65 * (2 * hp)
                h1 = 65 * (2 * hp + 1)
                nc.tensor.matmul(
                    ct0[:, :], va[:, kc, h0:h0 + 65], pt[:, 0, :],
                    start=(kc == 0), stop=(kc == KC - 1),
                )
                nc.tensor.matmul(
                    ct1[:, :], va[:, kc, h1:h1 + 65], pt[:, 1, :],
                    start=(kc == 0), stop=(kc == KC - 1),
                )
                for fn in sched.get((pidx, kc), ()):
                    fn()

            # ---- stash unnormalized ctx^T; per-pair 1/denom dance ----
            c0 = cp.tile([64, QR], bf16, tag=f"ctf{b}_{2 * hp}", name=f"ctf{b}_{2 * hp}")
            c1 = cp.tile([64, QR], bf16, tag=f"ctf{b}_{2 * hp + 1}", name=f"ctf{b}_{2 * hp + 1}")
            nc.scalar.copy(c0[:], ct0[0:64, :])
            nc.scalar.copy(c1[:], ct1[0:64, :])
            den = rp.tile([1, 2, QR], f32, tag="den")
            nc.vector.tensor_copy(den[0:1, 0, :], ct0[64:65, :])
            nc.vector.tensor_copy(den[0:1, 1, :], ct1[64:65, :])
            rec = rp.tile([1, 2, QR], f32, tag="rec")
            nc.vector.reciprocal_approx_fast(
                rec[0:1, :, :].rearrange("p a b -> p (a b)"),
                den[0:1, :, :].rearrange("p a b -> p (a b)"),
            )
            # broadcast 1/den across 64 partitions with a K=1 fp32 matmul
            # (ones^T @ rec) -- replaces a ~3.5us SWDGE DMA broadcast
            bcp = ps_big.tile([128, 2, 512], f32, tag="sc", name="auxb")
            for j, cj in ((0, c0), (1, c1)):
                nc.tensor.matmul(
                    bcp[0:64, j, :], ones32_sb[0:1, :], rec[0:1, j, :],
                    start=True, stop=True,
                )
                nc.vector.tensor_mul(cj[:], cj[:], bcp[0:64, j, :])
            ctf[b][2 * hp] = c0
            ctf[b][2 * hp + 1] = c1

        out_proj(1)

    nc.compile()
    _BUILT["nc"] = nc
    return nc


def host_inputs(x, G, Wq, Wk, Wv, Wo, bo, b_extra):
    """Build the per-core input maps (layout prep + query-row sharding)."""
    import ml_dtypes

    f = np.float32
    bf = ml_dtypes.bfloat16
    x = np.asarray(x, f)
    G = np.asarray(G, f)
    xt = np.ascontiguousarray(x.transpose(0, 2, 1)).reshape(B, 2, 128, S).astype(bf)
    wq = np.ascontiguousarray(np.asarray(Wq, f).T * SCALE).reshape(2, 128, 256).astype(bf)
    wk = np.ascontiguousarray(np.asarray(Wk, f).T).reshape(2, 128, 256).astype(bf)
    wv = np.ascontiguousarray(np.asarray(Wv, f).T).reshape(2, 128, 256).astype(bf)
    wo = np.ascontiguousarray(np.asarray(Wo, f).T).reshape(H, 64, 256).astype(bf)
    bias = (np.asarray(bo, f) + np.asarray(b_extra, f)).reshape(1, 256).astype(bf)

    shared = {"xt": xt, "wq": wq, "wk": wk, "wv": wv, "wo": wo, "bias": bias}
    in_maps = []
    for c in range(NCORES):
        q0 = c * QR
        m = dict(shared)
        # host-side transpose to gt[p, kc, q] = G[q0+q, kc*128+p]
        gc = G[q0:q0 + QR, :].T.reshape(KC, 128, QR)
        m["g"] = np.ascontiguousarray(gc.transpose(1, 0, 2)).astype(np.float16)
        m["xqt"] = np.ascontiguousarray(xt[:, :, :, q0:q0 + QR])
        in_maps.append(m)
    return in_maps


def run(in_maps, trace=False):
    from concourse.bass_utils import run_bass_kernel_spmd

    nc = build_bass()
    return run_bass_kernel_spmd(nc, in_maps, core_ids=list(range(NCORES)), trace=trace)


def kernel(x, G, Wq, Wk, Wv, Wo, bo, b_extra):
    in_maps = host_inputs(x, G, Wq, Wk, Wv, Wo, bo, b_extra)
    res = run(in_maps, trace=False)
    out = np.concatenate([res.results[c]["out"] for c in range(NCORES)], axis=1)
    return out.astype(np.float32)


# revision 23
# speedup vs baseline: 1.1064x; 1.1064x over previous
"""MultiHead HGNN attention (B=2, S=4096, D=256, H=4) on 8 TRN2 NeuronCores.

Sharding: query rows split 8 ways (512 rows/core); every core computes all
batches/heads for its query block. Scores are built k-major (scores^T); G^T is
pre-transposed on the HOST (free) and DMA'd straight into SBUF as fp16.
Softmax denominators ride as ones-columns in the V operand; probs are fp16.

The mul-by-G + exp elementwise stage over 16.7M scores/core is split across
THREE engines per key chunk (pattern per 32-kc pair, tuned so DVE/ACT/Pool all
land ~109us):
  path a (x21): fused custom DVE op  i16 = max(s*g*A + B, 0)  writing the
          int16 bit pattern of fp16(e^(s*g)) (Schraudolph); PSUM-sourced.
  path d (x10): ACT drains scores PSUM->fp16, Pool (GpSimd) does the g-mul
          as a plain tensor_tensor mult (the only elementwise opcode walrus
          accepts on Pool; Pool also has no PSUM port, hence the drain), ACT
          does the relu(x*A+B)->i16 Schraudolph tail.
The softmax normalization divides out Schraudolph's common-mode error.

Startup: DMA order is wk, wq, xqt, xt[b0], wv, gt[0:2], gt[2:8], gt[8:32],
xt[b1] so the Q/K builds gate on ~3MB not 10MB. Only kt(0,0)+q+va(0,sg0..5)
build before the main loop; the other K/V builds are injected at fixed kc
positions inside earlier pairs' loops (they borrow scores-PSUM slots briefly
instead of serializing 48 rounds through 3 slots up front).

Pair-end softmax denominators: 1/den is broadcast across 64 partitions with a
K=1 fp32 matmul into PSUM (ones^T @ rec) instead of the previous ~3.5us SWDGE
DMA broadcast that stalled DVE at every pair boundary.
"""

import contextlib
import ctypes
import sys
import types

import numpy as np

sys.path.insert(0, "/opt/trn_rl_repo")

SCHRA_A = 1024.0 / float(np.log(2.0))   # 1477.3199 = 2^10 * log2(e)
SCHRA_B = 15360.0 - 100.0               # fp16 exponent bias<<10, sigma tuned on sim
# per-kc elementwise path pattern over the 32 key chunks of each (b,hp) pair:
# 'd' (ACT drain + Pool mul + ACT relu) every 3rd chunk, 'a' (fused DVE) rest
PATTERN32 = "".join("d" if (i % 3 == 1 and i < 29) else "a" for i in range(32))
assert len(PATTERN32) == 32


def _install_axon_hooks():
    """The agent image's antenv lacks axon_hooks; provide it so bass_utils can
    NTFF-profile under axon. Harmless when profiling is never requested."""
    if "antenv.axon_hooks" in sys.modules:
        return
    try:
        import antenv
    except ImportError:
        return
    mod = types.ModuleType("antenv.axon_hooks")
    holder = {}
    mod.set_axon_ntff_profile_hook = lambda h: holder.__setitem__("h", h)
    mod.get_axon_ntff_profile_hook = lambda: holder.get("h")
    sys.modules["antenv.axon_hooks"] = mod
    antenv.axon_hooks = mod
    try:
        lib = ctypes.CDLL("/opt/axon/libaxon_pjrt.so")
    except OSError:
        return
    if not hasattr(lib, "axon_start_nrt_profile"):
        return
    lib.axon_start_nrt_profile.argtypes = [ctypes.POINTER(ctypes.c_int64), ctypes.c_size_t]
    lib.axon_start_nrt_profile.restype = ctypes.c_int64
    lib.axon_stop_nrt_profile.argtypes = [ctypes.c_char_p]
    lib.axon_stop_nrt_profile.restype = ctypes.c_int64

    @contextlib.contextmanager
    def _hook(output_dir, device_ids):
        import jax

        jax.devices()
        if device_ids:
            ids = (ctypes.c_int64 * len(device_ids))(*device_ids)
            rc = lib.axon_start_nrt_profile(ids, len(device_ids))
        else:
            rc = lib.axon_start_nrt_profile(None, 0)
        if rc != 0:
            raise RuntimeError(f"axon_start_nrt_profile rc={rc}")
        try:
            yield
        finally:
            n = lib.axon_stop_nrt_profile(str(output_dir).encode())
            print(f"profile: {n} file(s) written to {output_dir}")

    mod.set_axon_ntff_profile_hook(_hook)


_install_axon_hooks()

B, S, D, H, HD = 2, 4096, 256, 4, 64
NCORES = 8
QR = S // NCORES          # 512 query rows per core
KC = S // 128             # 32 key chunks of 128
SCALE = 1.0 / np.sqrt(HD)

_BUILT = {}


def _register_schra_op():
    """Register the fused (Src0*Src1*C0 + C1 clamped at 0 -> int16) DVE op."""
    from concourse import dve_ops
    from concourse.dve_spec import Spec, Src0, Src1, C0, C1, Zero, maxx, lower, _has_src1
    from concourse.dve_uop import DveOpSpec

    if "schra_op" in _BUILT:
        return _BUILT["schra_op"]
    name = "SCHRA_MULADD_ANT"
    for existing in dve_ops.OPS:
        if existing.name == name:  # re-import in the same process
            _BUILT["schra_op"] = existing
            return existing
    spec = Spec(
        body=maxx(Src0 * Src1 * C0 + C1, Zero),
        reference=lambda in0, in1, s0, s1, imm2: np.maximum(
            in0.astype(np.float32) * in1.astype(np.float32) * s0 + s1, 0.0
        ).astype(np.float32),
    )
    row = dve_ops._CUSTOM_DVE_ROW_BASE + len(dve_ops.OPS)
    shas = {}
    for ver in ("v3", "v4"):
        s = DveOpSpec(name=name, opcode=row, uops=lower(spec, ver=ver), rd1_en=_has_src1(spec))
        shas[ver] = s.sha(ver)
    op = dve_ops.DveOp(name, spec, subdim=False, uops_sha=shas)
    dve_ops.OPS.append(op)
    dve_ops.CUSTOM_DVE_SPECS[name] = spec
    dve_ops._SUB_OPCODE_FOR_NAME[name] = row
    _BUILT["schra_op"] = op
    return op


def build_bass():
    if "nc" in _BUILT:
        return _BUILT["nc"]

    import concourse.tile as tile
    from concourse import bacc, mybir

    f32, bf16, f16, i16 = (
        mybir.dt.float32, mybir.dt.bfloat16, mybir.dt.float16, mybir.dt.int16,
    )
    f32r = mybir.dt.float32r
    af = mybir.ActivationFunctionType
    alu = mybir.AluOpType
    schra = _register_schra_op()

    nc = bacc.Bacc("TRN2", target_bir_lowering=False, debug=False, num_devices=NCORES)

    xt_in = nc.dram_tensor("xt", [B, 2, 128, S], bf16, kind="ExternalInput")
    xqt_in = nc.dram_tensor("xqt", [B, 2, 128, QR], bf16, kind="ExternalInput")
    g_in = nc.dram_tensor("g", [128, KC, QR], f16, kind="ExternalInput")
    wq_in = nc.dram_tensor("wq", [2, 128, 256], bf16, kind="ExternalInput")
    wk_in = nc.dram_tensor("wk", [2, 128, 256], bf16, kind="ExternalInput")
    wv_in = nc.dram_tensor("wv", [2, 128, 256], bf16, kind="ExternalInput")
    wo_in = nc.dram_tensor("wo", [H, 64, 256], bf16, kind="ExternalInput")
    bias_in = nc.dram_tensor("bias", [1, 256], bf16, kind="ExternalInput")
    out_dram = nc.dram_tensor("out", [B, QR, 256], f32, kind="ExternalOutput")

    with tile.TileContext(nc) as tc, contextlib.ExitStack() as ctx:
        cp = ctx.enter_context(tc.tile_pool(name="const", bufs=1))
        # 3 slots x 2 banks for scores / KVQ staging / out-proj / rec-broadcast
        ps_big = ctx.enter_context(tc.tile_pool(name="ps_big", bufs=3, space="PSUM"))
        ps_ct = ctx.enter_context(tc.tile_pool(name="ps_ct", bufs=1, space="PSUM"))

        # ---- weights, DMA'd in dependency order: wk/wq gate the K/Q builds
        wk_sb = cp.tile([128, 2, 256], bf16, tag="wk")
        wq_sb = cp.tile([128, 2, 256], bf16, tag="wq")
        wv_sb = cp.tile([128, 2, 256], bf16, tag="wv")
        bias_sb = cp.tile([1, 256], bf16, tag="bias")
        ones_sb = cp.tile([1, 128], bf16, tag="ones")
        ones32_sb = cp.tile([1, 64], f32, tag="ones32")
        schrab_sb = cp.tile([128, 1], f32, tag="schrab")
        nc.gpsimd.memset(schrab_sb[:], SCHRA_B)
        nc.gpsimd.memset(ones_sb[:], 1.0)
        nc.gpsimd.memset(ones32_sb[:], 1.0)
        for ic in range(2):
            nc.sync.dma_start(wk_sb[:, ic, :], wk_in[ic])
            nc.sync.dma_start(wq_sb[:, ic, :], wq_in[ic])

        # xqt before xt so the Q projection (which gates the first scores)
        # isn't queued behind 4MB of x^T
        xqt_sb = cp.tile([128, B, 2, QR], bf16, tag="xqt")
        for b in range(B):
            for ic in range(2):
                nc.sync.dma_start(xqt_sb[:, b, ic, :], xqt_in[b, ic])

        xt_sb = [[cp.tile([128, S], bf16, tag=f"xt{b}{ic}", name=f"xt{b}{ic}") for ic in range(2)] for b in range(B)]
        for ic in range(2):
            nc.sync.dma_start(xt_sb[0][ic][:], xt_in[0, ic])
        for ic in range(2):
            nc.sync.dma_start(wv_sb[:, ic, :], wv_in[ic])

        # G^T: host-pretransposed; first 2 kc land within ~10us so pair-0
        # elementwise never waits, bulk follows, then batch-1 x^T
        gt_sb = cp.tile([128, KC, QR], f16, tag="gt")
        nc.sync.dma_start(gt_sb[:, 0:2, :], g_in[:, 0:2, :])
        nc.sync.dma_start(gt_sb[:, 2:8, :], g_in[:, 2:8, :])
        for kp in range(1, 4):
            nc.sync.dma_start(gt_sb[:, kp * 8:(kp + 1) * 8, :], g_in[:, kp * 8:(kp + 1) * 8, :])
        for ic in range(2):
            nc.sync.dma_start(xt_sb[1][ic][:], xt_in[1, ic])
        wo_sb = []
        for h in range(H):
            t = cp.tile([64, 256], bf16, tag=f"wo{h}", name=f"wo{h}")
            nc.sync.dma_start(t[:], wo_in[h])
            wo_sb.append(t)
        nc.sync.dma_start(bias_sb[:], bias_in[:])

        qts = [[cp.tile([128, QR], bf16, tag=f"qt{b}{hp}", name=f"qt{b}{hp}") for hp in range(2)] for b in range(B)]

        vap = ctx.enter_context(tc.tile_pool(name="vap", bufs=2))
        stp = ctx.enter_context(tc.tile_pool(name="stp", bufs=4))
        ttp = ctx.enter_context(tc.tile_pool(name="ttp", bufs=4))
        pp = ctx.enter_context(tc.tile_pool(name="pp", bufs=5))
        otp = ctx.enter_context(tc.tile_pool(name="otp", bufs=2))
        rp = ctx.enter_context(tc.tile_pool(name="rp", bufs=2))

        ctf = [[None] * H for _ in range(B)]

        # va[keys, kc, 65h:65h+64] per batch, ones col at 65h+64 (denominators,
        # memset once up front; the per-sg copies skip those columns)
        vas = [vap.tile([128, KC, 260], f16, tag="va", name=f"va{b}") for b in range(B)]
        for b in range(B):
            nc.vector.memset(
                vas[b][:, :, :].rearrange("p k (h x) -> p k h x", h=4)[:, :, :, 64:65], 1.0
            )
        kts = [[cp.tile([128, S], bf16, tag=f"kt{b}{hp}", name=f"kt{b}{hp}") for hp in range(2)] for b in range(B)]

        def q_build(b, hp):
            aux = ps_big.tile([128, 2, 512], f32, tag="sc", name="auxq")
            for ic in range(2):
                nc.tensor.matmul(
                    aux[:, 0, :QR], wq_sb[:, ic, hp * 128:(hp + 1) * 128],
                    xqt_sb[:, b, ic, :], start=(ic == 0), stop=(ic == 1),
                )
            nc.vector.tensor_copy(qts[b][hp][:], aux[:, 0, :QR])

        def va_round(b, sg):
            va = vas[b]
            vps = ps_big.tile([128, 2, 512], f32, tag="sc", name="auxv")
            for j in range(2):
                kcj = sg * 2 + j
                for ic in range(2):
                    nc.tensor.matmul(
                        vps[:, j, 0:256],
                        xt_sb[b][ic][:, kcj * 128:(kcj + 1) * 128],
                        wv_sb[:, ic, :],
                        start=(ic == 0), stop=(ic == 1),
                    )
            nc.scalar.copy(
                va[:, sg * 2:(sg + 1) * 2, :].rearrange("p k (h x) -> p k h x", h=4)[:, :, :, 0:64],
                vps[:, 0:2, 0:256].rearrange("p j (h x) -> p j h x", h=4),
            )

        def kt_round(b, hp, sc4):
            kt = kts[b][hp]
            auxk = ps_big.tile([128, 2, 512], f32, tag="sc", name="auxk")
            for half in range(2):
                sc8 = sc4 * 2 + half
                for ic in range(2):
                    nc.tensor.matmul(
                        auxk[:, half, :], wk_sb[:, ic, hp * 128:(hp + 1) * 128],
                        xt_sb[b][ic][:, sc8 * 512:(sc8 + 1) * 512],
                        start=(ic == 0), stop=(ic == 1),
                    )
            nc.scalar.copy(
                kt[:, sc4 * 1024:(sc4 + 1) * 1024],
                auxk[:, 0:2, :].rearrange("p a b -> p (a b)"),
            )

        def out_proj(b):
            for qs in range(QR // 128):
                op = ps_big.tile([128, 2, 512], f32, tag="sc", name="auxo")
                for h in range(H):
                    nc.tensor.matmul(
                        op[:, 0, 0:256], ctf[b][h][:, qs * 128:(qs + 1) * 128],
                        wo_sb[h][:], start=(h == 0), stop=False,
                    )
                nc.tensor.matmul(op[:, 0, 0:256], ones_sb[0:1, :], bias_sb[0:1, :], start=False, stop=True)
                ot = otp.tile([128, 256], f32, tag="ot")
                nc.vector.tensor_copy(ot[:], op[:, 0, 0:256])
                nc.sync.dma_start(out_dram[b, qs * 128:(qs + 1) * 128, :], ot[:])

        # ---- minimal upfront builds; the rest is injected into pair loops
        for b in range(B):
            for hp in range(2):
                q_build(b, hp)
        for sc4 in range(4):
            kt_round(0, 0, sc4)
        for sg in range(6):
            va_round(0, sg)

        sched = {}

        def add(pidx, kc, fn):
            sched.setdefault((pidx, kc), []).append(fn)

        for i, sg in enumerate(range(6, 16)):          # va(0) tail
            add(0, 1 + 2 * i, lambda s=sg: va_round(0, s))
        for i, s4 in enumerate(range(4)):              # kt(0,1)
            add(0, 21 + 2 * i, lambda s=s4: kt_round(0, 1, s))
        for sg in range(16):                           # va(1)
            add(1, 1 + 2 * sg, lambda s=sg: va_round(1, s))
        for i, s4 in enumerate(range(4)):              # kt(1,0)
            add(1, 22 + 2 * i, lambda s=s4: kt_round(1, 0, s))
        for i, s4 in enumerate(range(4)):              # kt(1,1)
            add(2, 2 + 2 * i, lambda s=s4: kt_round(1, 1, s))
        add(2, 8, lambda: out_proj(0))

        for pidx, (b, hp) in enumerate([(0, 0), (0, 1), (1, 0), (1, 1)]):
            va = vas[b]
            qt = qts[b][hp]
            kt = kts[b][hp]

            # ---- main loop over key chunks ----
            ct0 = ps_ct.tile([65, QR], f32, tag="ct0")
            ct1 = ps_ct.tile([65, QR], f32, tag="ct1")
            for kc in range(KC):
                scp = ps_big.tile([128, 2, QR], f32, tag="sc", name="scp")
                nc.tensor.matmul(
                    scp[:, 0, :], kt[0:64, kc * 128:(kc + 1) * 128], qt[0:64, :],
                    start=True, stop=True, tile_position=(0, 0),
                )
                nc.tensor.matmul(
                    scp[:, 1, :], kt[64:128, kc * 128:(kc + 1) * 128], qt[64:128, :],
                    start=True, stop=True, tile_position=(64, 0),
                )
                pt = pp.tile([128, 2, QR], f16, tag="pt")
                path = PATTERN32[kc]
                gtb = gt_sb[:, kc:kc + 1, :].broadcast_to([128, 2, QR])
                if path == "a":
                    # fused: i16 bits of fp16(e^(s*g)) straight from PSUM
                    nc.vector._custom_dve(
                        schra,
                        out=pt[:, :, :].bitcast(i16),
                        in0=scp[:, :, :],
                        in1=gtb,
                        s0=SCHRA_A, s1=SCHRA_B,
                    )
                else:
                    # staged per head so the drain->Pool-mul->relu chain is
                    # ~2.5us not ~5us: each head's ctx matmul waits only on
                    # its own half, keeping the in-order PE queue moving
                    sc16 = stp.tile([128, 2, QR], f16, tag="sc16")
                    tt = ttp.tile([128, 2, QR], f16, tag="tt")
                    for j in range(2):
                        nc.scalar.copy(sc16[:, j, :], scp[:, j, :])
                        nc.gpsimd.tensor_mul(tt[:, j, :], sc16[:, j, :], gt_sb[:, kc, :])
                        # Schraudolph tail on ACT: relu(t*A+B) -> i16
                        nc.scalar.activation(
                            pt[:, j, :].bitcast(i16), tt[:, j, :],
                            af.Relu, scale=SCHRA_A, bias=schrab_sb[:, :],
                        )
                h0 = 65 * (2 * hp)
                h1 = 65 * (2 * hp + 1)
                nc.tensor.matmul(
                    ct0[:, :], va[:, kc, h0:h0 + 65], pt[:, 0, :],
                    start=(kc == 0), stop=(kc == KC - 1),
                )
                nc.tensor.matmul(
                    ct1[:, :], va[:, kc, h1:h1 + 65], pt[:, 1, :],
                    start=(kc == 0), stop=(kc == KC - 1),
                )
                for fn in sched.get((pidx, kc), ()):
                    fn()

            # ---- stash unnormalized ctx^T; per-pair 1/denom dance ----
            c0 = cp.tile([64, QR], bf16, tag=f"ctf{b}_{2 * hp}", name=f"ctf{b}_{2 * hp}")
            c1 = cp.tile([64, QR], bf16, tag=f"ctf{b}_{2 * hp + 1}", name=f"ctf{b}_{2 * hp + 1}")
            nc.scalar.copy(c0[:], ct0[0:64, :])
            nc.scalar.copy(c1[:], ct1[0:64, :])
            den = rp.tile([1, 2, QR], f32, tag="den")
            nc.vector.tensor_copy(den[0:1, 0, :], ct0[64:65, :])
            nc.vector.tensor_copy(den[0:1, 1, :], ct1[64:65, :])
            rec = rp.tile([1, 2, QR], f32, tag="rec")
            nc.vector.reciprocal_approx_fast(
                rec[0:1, :, :].rearrange("p a b -> p (a b)"),
                den[0:1, :, :].rearrange("p a b -> p (a b)"),
            )
            if pidx < 3:
                # mid-kernel: 1/den broadcast via DMA (no engine cost) and the
                # normalize on Pool, which idles around pair ends -- nothing
                # urgent waits on ctf until out_proj many steps later
                for j, cj in ((0, c0), (1, c1)):
                    bcb = rp.tile([64, QR], f32, tag="bcb")
                    nc.sync.dma_start(
                        bcb[:, :],
                        rec[0:1, j, :].rearrange("p (o q) -> p o q", o=1).broadcast_to([1, 64, QR]),
                    )
                    nc.gpsimd.tensor_mul(cj[:], cj[:], bcb[:, :])
            else:
                # tail pair: broadcast via a K=1 fp32 matmul into PSUM + DVE
                # mul -- ~2.5us shorter critical path into out_proj(1)
                bcp = ps_big.tile([128, 2, 512], f32, tag="sc", name="auxb")
                for j, cj in ((0, c0), (1, c1)):
                    nc.tensor.matmul(
                        bcp[0:64, j, :], ones32_sb[0:1, :], rec[0:1, j, :],
                        start=True, stop=True,
                    )
                    nc.vector.tensor_mul(cj[:], cj[:], bcp[0:64, j, :])
            ctf[b][2 * hp] = c0
            ctf[b][2 * hp + 1] = c1

        out_proj(1)

    nc.compile()
    _BUILT["nc"] = nc
    return nc


def host_inputs(x, G, Wq, Wk, Wv, Wo, bo, b_extra):
    """Build the per-core input maps (layout prep + query-row sharding)."""
    import ml_dtypes

    f = np.float32
    bf = ml_dtypes.bfloat16
    x = np.asarray(x, f)
    G = np.asarray(G, f)
    xt = np.ascontiguousarray(x.transpose(0, 2, 1)).reshape(B, 2, 128, S).astype(bf)
    wq = np.ascontiguousarray(np.asarray(Wq, f).T * SCALE).reshape(2, 128, 256).astype(bf)
    wk = np.ascontiguousarray(np.asarray(Wk, f).T).reshape(2, 128, 256).astype(bf)
    wv = np.ascontiguousarray(np.asarray(Wv, f).T).reshape(2, 128, 256).astype(bf)
    wo = np.ascontiguousarray(np.asarray(Wo, f).T).reshape(H, 64, 256).astype(bf)
    bias = (np.asarray(bo, f) + np.asarray(b_extra, f)).reshape(1, 256).astype(bf)

    shared = {"xt": xt, "wq": wq, "wk": wk, "wv": wv, "wo": wo, "bias": bias}
    in_maps = []
    for c in range(NCORES):
        q0 = c * QR
        m = dict(shared)
        # host-side transpose to gt[p, kc, q] = G[q0+q, kc*128+p]
        gc = G[q0:q0 + QR, :].T.reshape(KC, 128, QR)
        m["g"] = np.ascontiguousarray(gc.transpose(1, 0, 2)).astype(np.float16)
        m["xqt"] = np.ascontiguousarray(xt[:, :, :, q0:q0 + QR])
        in_maps.append(m)
    return in_maps


def run(in_maps, trace=False):
    from concourse.bass_utils import run_bass_kernel_spmd

    nc = build_bass()
    return run_bass_kernel_spmd(nc, in_maps, core_ids=list(range(NCORES)), trace=trace)


def kernel(x, G, Wq, Wk, Wv, Wo, bo, b_extra):
    in_maps = host_inputs(x, G, Wq, Wk, Wv, Wo, bo, b_extra)
    res = run(in_maps, trace=False)
    out = np.concatenate([res.results[c]["out"] for c in range(NCORES)], axis=1)
    return out.astype(np.float32)


# revision 28
# speedup vs baseline: 1.1431x; 1.0331x over previous
"""MultiHead HGNN attention (B=2, S=4096, D=256, H=4) on 8 TRN2 NeuronCores.

Sharding: query rows split 8 ways (512 rows/core); every core computes all
batches/heads for its query block. Scores are built k-major (scores^T); G^T is
pre-transposed on the HOST (free) and DMA'd straight into SBUF as fp16.
Softmax denominators ride as ones-columns in the V operand; probs are fp16.

The mul-by-G + exp elementwise stage over 16.7M scores/core is split across
THREE engines per key chunk (pattern per 32-kc pair, tuned so DVE/ACT/Pool all
land ~109us):
  path a (x21): fused custom DVE op  i16 = max(s*g*A + B, 0)  writing the
          int16 bit pattern of fp16(e^(s*g)) (Schraudolph); PSUM-sourced.
  path d (x10): ACT drains scores PSUM->fp16, Pool (GpSimd) does the g-mul
          as a plain tensor_tensor mult (the only elementwise opcode walrus
          accepts on Pool; Pool also has no PSUM port, hence the drain), ACT
          does the relu(x*A+B)->i16 Schraudolph tail.
The softmax normalization divides out Schraudolph's common-mode error.

Startup: DMA order is wk, wq, xqt, xt[b0], wv, gt[0:2], gt[2:8], gt[8:32],
xt[b1] so the Q/K builds gate on ~3MB not 10MB. Only kt(0,0)+q+va(0,sg0..5)
build before the main loop; the other K/V builds are injected at fixed kc
positions inside earlier pairs' loops (they borrow scores-PSUM slots briefly
instead of serializing 48 rounds through 3 slots up front).

Pair-end softmax denominators: 1/den is broadcast across 64 partitions with a
K=1 fp32 matmul into PSUM (ones^T @ rec) instead of the previous ~3.5us SWDGE
DMA broadcast that stalled DVE at every pair boundary.
"""

import contextlib
import ctypes
import sys
import types

import numpy as np

sys.path.insert(0, "/opt/trn_rl_repo")

SCHRA_A = 1024.0 / float(np.log(2.0))   # 1477.3199 = 2^10 * log2(e)
SCHRA_B = 15360.0 - 100.0               # fp16 exponent bias<<10, sigma tuned on sim
# per-kc elementwise path pattern over the 32 key chunks of each (b,hp) pair:
# 'd' (ACT drain + Pool mul + ACT relu) every 3rd chunk, 'a' (fused DVE) rest.
# No 'd' in the last 8 chunks: the pair-end ct release must not queue behind
# leftover ACT/Pool work or the next pair's ctx matmuls stall the PE FIFO.
PATTERN32 = "".join("d" if (i % 3 == 1 and i < 24) else "a" for i in range(32))
assert len(PATTERN32) == 32


def _install_axon_hooks():
    """The agent image's antenv lacks axon_hooks; provide it so bass_utils can
    NTFF-profile under axon. Harmless when profiling is never requested."""
    if "antenv.axon_hooks" in sys.modules:
        return
    try:
        import antenv
    except ImportError:
        return
    mod = types.ModuleType("antenv.axon_hooks")
    holder = {}
    mod.set_axon_ntff_profile_hook = lambda h: holder.__setitem__("h", h)
    mod.get_axon_ntff_profile_hook = lambda: holder.get("h")
    sys.modules["antenv.axon_hooks"] = mod
    antenv.axon_hooks = mod
    try:
        lib = ctypes.CDLL("/opt/axon/libaxon_pjrt.so")
    except OSError:
        return
    if not hasattr(lib, "axon_start_nrt_profile"):
        return
    lib.axon_start_nrt_profile.argtypes = [ctypes.POINTER(ctypes.c_int64), ctypes.c_size_t]
    lib.axon_start_nrt_profile.restype = ctypes.c_int64
    lib.axon_stop_nrt_profile.argtypes = [ctypes.c_char_p]
    lib.axon_stop_nrt_profile.restype = ctypes.c_int64

    @contextlib.contextmanager
    def _hook(output_dir, device_ids):
        import jax

        jax.devices()
        if device_ids:
            ids = (ctypes.c_int64 * len(device_ids))(*device_ids)
            rc = lib.axon_start_nrt_profile(ids, len(device_ids))
        else:
            rc = lib.axon_start_nrt_profile(None, 0)
        if rc != 0:
            raise RuntimeError(f"axon_start_nrt_profile rc={rc}")
        try:
            yield
        finally:
            n = lib.axon_stop_nrt_profile(str(output_dir).encode())
            print(f"profile: {n} file(s) written to {output_dir}")

    mod.set_axon_ntff_profile_hook(_hook)


_install_axon_hooks()

B, S, D, H, HD = 2, 4096, 256, 4, 64
NCORES = 8
QR = S // NCORES          # 512 query rows per core
KC = S // 128             # 32 key chunks of 128
SCALE = 1.0 / np.sqrt(HD)

_BUILT = {}


def _register_schra_op():
    """Register the fused (Src0*Src1*C0 + C1 clamped at 0 -> int16) DVE op."""
    from concourse import dve_ops
    from concourse.dve_spec import Spec, Src0, Src1, C0, C1, Zero, maxx, lower, _has_src1
    from concourse.dve_uop import DveOpSpec

    if "schra_op" in _BUILT:
        return _BUILT["schra_op"]
    name = "SCHRA_MULADD_ANT"
    for existing in dve_ops.OPS:
        if existing.name == name:  # re-import in the same process
            _BUILT["schra_op"] = existing
            return existing
    spec = Spec(
        body=maxx(Src0 * Src1 * C0 + C1, Zero),
        reference=lambda in0, in1, s0, s1, imm2: np.maximum(
            in0.astype(np.float32) * in1.astype(np.float32) * s0 + s1, 0.0
        ).astype(np.float32),
    )
    row = dve_ops._CUSTOM_DVE_ROW_BASE + len(dve_ops.OPS)
    shas = {}
    for ver in ("v3", "v4"):
        s = DveOpSpec(name=name, opcode=row, uops=lower(spec, ver=ver), rd1_en=_has_src1(spec))
        shas[ver] = s.sha(ver)
    op = dve_ops.DveOp(name, spec, subdim=False, uops_sha=shas)
    dve_ops.OPS.append(op)
    dve_ops.CUSTOM_DVE_SPECS[name] = spec
    dve_ops._SUB_OPCODE_FOR_NAME[name] = row
    _BUILT["schra_op"] = op
    return op


def build_bass():
    if "nc" in _BUILT:
        return _BUILT["nc"]

    import concourse.tile as tile
    from concourse import bacc, mybir

    f32, bf16, f16, i16 = (
        mybir.dt.float32, mybir.dt.bfloat16, mybir.dt.float16, mybir.dt.int16,
    )
    f32r = mybir.dt.float32r
    af = mybir.ActivationFunctionType
    alu = mybir.AluOpType
    schra = _register_schra_op()

    nc = bacc.Bacc("TRN2", target_bir_lowering=False, debug=False, num_devices=NCORES)

    xt_in = nc.dram_tensor("xt", [B, 2, 128, S], bf16, kind="ExternalInput")
    xqt_in = nc.dram_tensor("xqt", [B, 2, 128, QR], bf16, kind="ExternalInput")
    g_in = nc.dram_tensor("g", [128, KC, QR], f16, kind="ExternalInput")
    wq_in = nc.dram_tensor("wq", [2, 128, 256], bf16, kind="ExternalInput")
    wk_in = nc.dram_tensor("wk", [2, 128, 256], bf16, kind="ExternalInput")
    wv_in = nc.dram_tensor("wv", [2, 128, 256], bf16, kind="ExternalInput")
    wo_in = nc.dram_tensor("wo", [H, 64, 256], bf16, kind="ExternalInput")
    bias_in = nc.dram_tensor("bias", [1, 256], bf16, kind="ExternalInput")
    out_dram = nc.dram_tensor("out", [B, QR, 256], f32, kind="ExternalOutput")

    with tile.TileContext(nc) as tc, contextlib.ExitStack() as ctx:
        cp = ctx.enter_context(tc.tile_pool(name="const", bufs=1))
        # 3 slots x 2 banks for scores / KVQ staging / out-proj / rec-broadcast
        ps_big = ctx.enter_context(tc.tile_pool(name="ps_big", bufs=3, space="PSUM"))
        ps_ct = ctx.enter_context(tc.tile_pool(name="ps_ct", bufs=1, space="PSUM"))

        # ---- weights, DMA'd in dependency order: wk/wq gate the K/Q builds
        wk_sb = cp.tile([128, 2, 256], bf16, tag="wk")
        wq_sb = cp.tile([128, 2, 256], bf16, tag="wq")
        wv_sb = cp.tile([128, 2, 256], bf16, tag="wv")
        bias_sb = cp.tile([1, 256], bf16, tag="bias")
        ones_sb = cp.tile([1, 128], bf16, tag="ones")
        ones16_sb = cp.tile([1, 64], f16, tag="ones16")
        schrab_sb = cp.tile([128, 1], f32, tag="schrab")
        nc.gpsimd.memset(schrab_sb[:], SCHRA_B)
        nc.gpsimd.memset(ones_sb[:], 1.0)
        nc.gpsimd.memset(ones16_sb[:], 1.0)
        for ic in range(2):
            nc.sync.dma_start(wk_sb[:, ic, :], wk_in[ic])
            nc.sync.dma_start(wq_sb[:, ic, :], wq_in[ic])

        # xqt before xt so the Q projection (which gates the first scores)
        # isn't queued behind 4MB of x^T
        xqt_sb = cp.tile([128, B, 2, QR], bf16, tag="xqt")
        for b in range(B):
            for ic in range(2):
                nc.sync.dma_start(xqt_sb[:, b, ic, :], xqt_in[b, ic])

        xt_sb = [[cp.tile([128, S], bf16, tag=f"xt{b}{ic}", name=f"xt{b}{ic}") for ic in range(2)] for b in range(B)]
        # batch-0 x^T in column halves so the first K/V builds gate on 1MB
        for half in range(2):
            for ic in range(2):
                nc.sync.dma_start(
                    xt_sb[0][ic][:, half * 2048:(half + 1) * 2048],
                    xt_in[0, ic, :, half * 2048:(half + 1) * 2048],
                )
        for ic in range(2):
            nc.sync.dma_start(wv_sb[:, ic, :], wv_in[ic])

        # G^T: host-pretransposed; first 2 kc land within ~10us so pair-0
        # elementwise never waits, bulk follows, then batch-1 x^T
        gt_sb = cp.tile([128, KC, QR], f16, tag="gt")
        nc.sync.dma_start(gt_sb[:, 0:2, :], g_in[:, 0:2, :])
        nc.sync.dma_start(gt_sb[:, 2:8, :], g_in[:, 2:8, :])
        for kp in range(1, 4):
            nc.sync.dma_start(gt_sb[:, kp * 8:(kp + 1) * 8, :], g_in[:, kp * 8:(kp + 1) * 8, :])
        for ic in range(2):
            nc.sync.dma_start(xt_sb[1][ic][:], xt_in[1, ic])
        wo_sb = []
        for h in range(H):
            t = cp.tile([64, 256], bf16, tag=f"wo{h}", name=f"wo{h}")
            nc.sync.dma_start(t[:], wo_in[h])
            wo_sb.append(t)
        nc.sync.dma_start(bias_sb[:], bias_in[:])

        qts = [[cp.tile([128, QR], bf16, tag=f"qt{b}{hp}", name=f"qt{b}{hp}") for hp in range(2)] for b in range(B)]

        vap = ctx.enter_context(tc.tile_pool(name="vap", bufs=2))
        stp = ctx.enter_context(tc.tile_pool(name="stp", bufs=4))
        ttp = ctx.enter_context(tc.tile_pool(name="ttp", bufs=4))
        pp = ctx.enter_context(tc.tile_pool(name="pp", bufs=5))
        otp = ctx.enter_context(tc.tile_pool(name="otp", bufs=2))
        rp = ctx.enter_context(tc.tile_pool(name="rp", bufs=2))

        ctf = [[None] * H for _ in range(B)]

        # va[keys, kc, 65h:65h+64] per batch, ones col at 65h+64 (denominators,
        # memset once up front; the per-sg copies skip those columns)
        vas = [vap.tile([128, KC, 260], f16, tag="va", name=f"va{b}") for b in range(B)]
        for b in range(B):
            nc.vector.memset(
                vas[b][:, :, :].rearrange("p k (h x) -> p k h x", h=4)[:, :, :, 64:65], 1.0
            )
        kts = [[cp.tile([128, S], bf16, tag=f"kt{b}{hp}", name=f"kt{b}{hp}") for hp in range(2)] for b in range(B)]

        def q_build(b, hp):
            aux = ps_big.tile([128, 2, 512], f32, tag="sc", name="auxq")
            for ic in range(2):
                nc.tensor.matmul(
                    aux[:, 0, :QR], wq_sb[:, ic, hp * 128:(hp + 1) * 128],
                    xqt_sb[:, b, ic, :], start=(ic == 0), stop=(ic == 1),
                )
            nc.vector.tensor_copy(qts[b][hp][:], aux[:, 0, :QR])

        def va_round(b, sg):
            va = vas[b]
            vps = ps_big.tile([128, 2, 512], f32, tag="sc", name="auxv")
            for j in range(2):
                kcj = sg * 2 + j
                for ic in range(2):
                    nc.tensor.matmul(
                        vps[:, j, 0:256],
                        xt_sb[b][ic][:, kcj * 128:(kcj + 1) * 128],
                        wv_sb[:, ic, :],
                        start=(ic == 0), stop=(ic == 1),
                    )
            nc.scalar.copy(
                va[:, sg * 2:(sg + 1) * 2, :].rearrange("p k (h x) -> p k h x", h=4)[:, :, :, 0:64],
                vps[:, 0:2, 0:256].rearrange("p j (h x) -> p j h x", h=4),
            )

        def kt_round(b, hp, sc4):
            kt = kts[b][hp]
            auxk = ps_big.tile([128, 2, 512], f32, tag="sc", name="auxk")
            for half in range(2):
                sc8 = sc4 * 2 + half
                for ic in range(2):
                    nc.tensor.matmul(
                        auxk[:, half, :], wk_sb[:, ic, hp * 128:(hp + 1) * 128],
                        xt_sb[b][ic][:, sc8 * 512:(sc8 + 1) * 512],
                        start=(ic == 0), stop=(ic == 1),
                    )
            nc.scalar.copy(
                kt[:, sc4 * 1024:(sc4 + 1) * 1024],
                auxk[:, 0:2, :].rearrange("p a b -> p (a b)"),
            )

        def out_proj(b):
            for qs in range(QR // 128):
                op = ps_big.tile([128, 2, 512], f32, tag="sc", name="auxo")
                for h in range(H):
                    nc.tensor.matmul(
                        op[:, 0, 0:256], ctf[b][h][:, qs * 128:(qs + 1) * 128],
                        wo_sb[h][:], start=(h == 0), stop=False,
                    )
                nc.tensor.matmul(op[:, 0, 0:256], ones_sb[0:1, :], bias_sb[0:1, :], start=False, stop=True)
                ot = otp.tile([128, 256], f32, tag="ot")
                nc.vector.tensor_copy(ot[:], op[:, 0, 0:256])
                nc.sync.dma_start(out_dram[b, qs * 128:(qs + 1) * 128, :], ot[:])

        # ---- minimal upfront builds; the rest is injected into pair loops
        for b in range(B):
            for hp in range(2):
                q_build(b, hp)
        for sc4 in range(4):
            kt_round(0, 0, sc4)
        for sg in range(6):
            va_round(0, sg)

        sched = {}

        def add(pidx, kc, fn):
            sched.setdefault((pidx, kc), []).append(fn)

        for i, sg in enumerate(range(6, 16)):          # va(0) tail
            add(0, 1 + 2 * i, lambda s=sg: va_round(0, s))
        for i, s4 in enumerate(range(4)):              # kt(0,1)
            add(0, 21 + 2 * i, lambda s=s4: kt_round(0, 1, s))
        for sg in range(16):                           # va(1)
            add(1, 1 + 2 * sg, lambda s=sg: va_round(1, s))
        for i, s4 in enumerate(range(4)):              # kt(1,0)
            add(1, 22 + 2 * i, lambda s=s4: kt_round(1, 0, s))
        for i, s4 in enumerate(range(4)):              # kt(1,1)
            add(2, 2 + 2 * i, lambda s=s4: kt_round(1, 1, s))
        add(2, 8, lambda: out_proj(0))

        for pidx, (b, hp) in enumerate([(0, 0), (0, 1), (1, 0), (1, 1)]):
            va = vas[b]
            qt = qts[b][hp]
            kt = kts[b][hp]

            # ---- main loop over key chunks ----
            ct0 = ps_ct.tile([65, QR], f32, tag="ct0")
            ct1 = ps_ct.tile([65, QR], f32, tag="ct1")
            for kc in range(KC):
                scp = ps_big.tile([128, 2, QR], f32, tag="sc", name="scp")
                nc.tensor.matmul(
                    scp[:, 0, :], kt[0:64, kc * 128:(kc + 1) * 128], qt[0:64, :],
                    start=True, stop=True, tile_position=(0, 0),
                )
                nc.tensor.matmul(
                    scp[:, 1, :], kt[64:128, kc * 128:(kc + 1) * 128], qt[64:128, :],
                    start=True, stop=True, tile_position=(64, 0),
                )
                pt = pp.tile([128, 2, QR], f16, tag="pt")
                path = PATTERN32[kc]
                gtb = gt_sb[:, kc:kc + 1, :].broadcast_to([128, 2, QR])
                if path == "a":
                    # fused: i16 bits of fp16(e^(s*g)) straight from PSUM
                    nc.vector._custom_dve(
                        schra,
                        out=pt[:, :, :].bitcast(i16),
                        in0=scp[:, :, :],
                        in1=gtb,
                        s0=SCHRA_A, s1=SCHRA_B,
                    )
                else:
                    # staged per head so the drain->Pool-mul->relu chain is
                    # ~2.5us not ~5us: each head's ctx matmul waits only on
                    # its own half, keeping the in-order PE queue moving
                    sc16 = stp.tile([128, 2, QR], f16, tag="sc16")
                    tt = ttp.tile([128, 2, QR], f16, tag="tt")
                    for j in range(2):
                        nc.scalar.copy(sc16[:, j, :], scp[:, j, :])
                        nc.gpsimd.tensor_mul(tt[:, j, :], sc16[:, j, :], gt_sb[:, kc, :])
                        # Schraudolph tail on ACT: relu(t*A+B) -> i16
                        nc.scalar.activation(
                            pt[:, j, :].bitcast(i16), tt[:, j, :],
                            af.Relu, scale=SCHRA_A, bias=schrab_sb[:, :],
                        )
                h0 = 65 * (2 * hp)
                h1 = 65 * (2 * hp + 1)
                nc.tensor.matmul(
                    ct0[:, :], va[:, kc, h0:h0 + 65], pt[:, 0, :],
                    start=(kc == 0), stop=(kc == KC - 1),
                )
                nc.tensor.matmul(
                    ct1[:, :], va[:, kc, h1:h1 + 65], pt[:, 1, :],
                    start=(kc == 0), stop=(kc == KC - 1),
                )
                for fn in sched.get((pidx, kc), ()):
                    fn()

            # ---- stash unnormalized ctx^T; per-pair 64/denom dance ----
            # high_priority: the ct0/ct1 release (and the next pair's ctx
            # matmuls behind it) must not queue after leftover ACT/DVE work
            with tc.high_priority():
                c0 = cp.tile([64, QR], bf16, tag=f"ctf{b}_{2 * hp}", name=f"ctf{b}_{2 * hp}")
                c1 = cp.tile([64, QR], bf16, tag=f"ctf{b}_{2 * hp + 1}", name=f"ctf{b}_{2 * hp + 1}")
                nc.scalar.copy(c0[:], ct0[0:64, :])
                nc.scalar.copy(c1[:], ct1[0:64, :])
                den = rp.tile([1, 2, QR], f32, tag="den")
                nc.vector.tensor_copy(den[0:1, 0, :], ct0[64:65, :])
                nc.vector.tensor_copy(den[0:1, 1, :], ct1[64:65, :])
                rec = rp.tile([1, 2, QR], f32, tag="rec")
                nc.vector.reciprocal_approx_fast(
                    rec[0:1, :, :].rearrange("p a b -> p (a b)"),
                    den[0:1, :, :].rearrange("p a b -> p (a b)"),
                )
                # rec16 = f16(64/den): the x64 dodges f16 subnormals (1/den can
                # be ~6e-5); Wo is pre-divided by 64 on the host to compensate.
                rec16 = rp.tile([1, 2, QR], f16, tag="rec16")
                nc.scalar.mul(rec16[0:1, :, :].rearrange("p a b -> p (a b)"),
                              rec[0:1, :, :].rearrange("p a b -> p (a b)"), 64.0)
                # broadcast across 64 partitions with a K=1 f16 matmul (~0.2us)
                bcp = ps_big.tile([128, 2, 512], f32, tag="sc", name="auxb")
                for j, cj in ((0, c0), (1, c1)):
                    nc.tensor.matmul(
                        bcp[0:64, j, :], ones16_sb[0:1, :], rec16[0:1, j, :],
                        start=True, stop=True,
                    )
                    nc.vector.tensor_mul(cj[:], cj[:], bcp[0:64, j, :])
            ctf[b][2 * hp] = c0
            ctf[b][2 * hp + 1] = c1

        out_proj(1)

    nc.compile()
    _BUILT["nc"] = nc
    return nc


def host_inputs(x, G, Wq, Wk, Wv, Wo, bo, b_extra):
    """Build the per-core input maps (layout prep + query-row sharding)."""
    import ml_dtypes

    f = np.float32
    bf = ml_dtypes.bfloat16
    x = np.asarray(x, f)
    G = np.asarray(G, f)
    xt = np.ascontiguousarray(x.transpose(0, 2, 1)).reshape(B, 2, 128, S).astype(bf)
    wq = np.ascontiguousarray(np.asarray(Wq, f).T * SCALE).reshape(2, 128, 256).astype(bf)
    wk = np.ascontiguousarray(np.asarray(Wk, f).T).reshape(2, 128, 256).astype(bf)
    wv = np.ascontiguousarray(np.asarray(Wv, f).T).reshape(2, 128, 256).astype(bf)
    # Wo/64 compensates the 64/den normalization scale (see kernel pair-end)
    wo = np.ascontiguousarray(np.asarray(Wo, f).T / 64.0).reshape(H, 64, 256).astype(bf)
    bias = (np.asarray(bo, f) + np.asarray(b_extra, f)).reshape(1, 256).astype(bf)

    shared = {"xt": xt, "wq": wq, "wk": wk, "wv": wv, "wo": wo, "bias": bias}
    in_maps = []
    for c in range(NCORES):
        q0 = c * QR
        m = dict(shared)
        # host-side transpose to gt[p, kc, q] = G[q0+q, kc*128+p]
        gc = G[q0:q0 + QR, :].T.reshape(KC, 128, QR)
        m["g"] = np.ascontiguousarray(gc.transpose(1, 0, 2)).astype(np.float16)
        m["xqt"] = np.ascontiguousarray(xt[:, :, :, q0:q0 + QR])
        in_maps.append(m)
    return in_maps


def run(in_maps, trace=False):
    from concourse.bass_utils import run_bass_kernel_spmd

    nc = build_bass()
    return run_bass_kernel_spmd(nc, in_maps, core_ids=list(range(NCORES)), trace=trace)


def kernel(x, G, Wq, Wk, Wv, Wo, bo, b_extra):
    in_maps = host_inputs(x, G, Wq, Wk, Wv, Wo, bo, b_extra)
    res = run(in_maps, trace=False)
    out = np.concatenate([res.results[c]["out"] for c in range(NCORES)], axis=1)
    return out.astype(np.float32)


# revision 31
# speedup vs baseline: 1.1665x; 1.0205x over previous
"""MultiHead HGNN attention (B=2, S=4096, D=256, H=4) on 8 TRN2 NeuronCores.

Sharding: query rows split 8 ways (512 rows/core); every core computes all
batches/heads for its query block. Scores are built k-major (scores^T); G^T is
pre-transposed on the HOST (free) and DMA'd straight into SBUF as fp16.
Softmax denominators ride as ones-columns in the V operand; probs are fp16.

The mul-by-G + exp elementwise stage over 16.7M scores/core is split across
THREE engines per key chunk (pattern per 32-kc pair, tuned so DVE/ACT/Pool all
land ~109us):
  path a (x21): fused custom DVE op  i16 = max(s*g*A + B, 0)  writing the
          int16 bit pattern of fp16(e^(s*g)) (Schraudolph); PSUM-sourced.
  path d (x10): ACT drains scores PSUM->fp16, Pool (GpSimd) does the g-mul
          as a plain tensor_tensor mult (the only elementwise opcode walrus
          accepts on Pool; Pool also has no PSUM port, hence the drain), ACT
          does the relu(x*A+B)->i16 Schraudolph tail.
The softmax normalization divides out Schraudolph's common-mode error.

Startup: DMA order is wk, wq, xqt, xt[b0], wv, gt[0:2], gt[2:8], gt[8:32],
xt[b1] so the Q/K builds gate on ~3MB not 10MB. Only kt(0,0)+q+va(0,sg0..5)
build before the main loop; the other K/V builds are injected at fixed kc
positions inside earlier pairs' loops (they borrow scores-PSUM slots briefly
instead of serializing 48 rounds through 3 slots up front).

Pair-end softmax denominators: 1/den is broadcast across 64 partitions with a
K=1 fp32 matmul into PSUM (ones^T @ rec) instead of the previous ~3.5us SWDGE
DMA broadcast that stalled DVE at every pair boundary.
"""

import contextlib
import ctypes
import sys
import types

import numpy as np

sys.path.insert(0, "/opt/trn_rl_repo")

SCHRA_A = 1024.0 / float(np.log(2.0))   # 1477.3199 = 2^10 * log2(e)
SCHRA_B = 15360.0 - 100.0               # fp16 exponent bias<<10, sigma tuned on sim
# per-kc elementwise path pattern over the 32 key chunks of each (b,hp) pair:
# 'd' (ACT drain + Pool mul + ACT relu) every 3rd chunk, 'a' (fused DVE) rest.
# No 'd' in the last 8 chunks: the pair-end ct release must not queue behind
# leftover ACT/Pool work or the next pair's ctx matmuls stall the PE FIFO.
PATTERN32 = "".join("d" if (i % 3 == 1 and i < 24) else "a" for i in range(32))
assert len(PATTERN32) == 32


def _install_axon_hooks():
    """The agent image's antenv lacks axon_hooks; provide it so bass_utils can
    NTFF-profile under axon. Harmless when profiling is never requested."""
    if "antenv.axon_hooks" in sys.modules:
        return
    try:
        import antenv
    except ImportError:
        return
    mod = types.ModuleType("antenv.axon_hooks")
    holder = {}
    mod.set_axon_ntff_profile_hook = lambda h: holder.__setitem__("h", h)
    mod.get_axon_ntff_profile_hook = lambda: holder.get("h")
    sys.modules["antenv.axon_hooks"] = mod
    antenv.axon_hooks = mod
    try:
        lib = ctypes.CDLL("/opt/axon/libaxon_pjrt.so")
    except OSError:
        return
    if not hasattr(lib, "axon_start_nrt_profile"):
        return
    lib.axon_start_nrt_profile.argtypes = [ctypes.POINTER(ctypes.c_int64), ctypes.c_size_t]
    lib.axon_start_nrt_profile.restype = ctypes.c_int64
    lib.axon_stop_nrt_profile.argtypes = [ctypes.c_char_p]
    lib.axon_stop_nrt_profile.restype = ctypes.c_int64

    @contextlib.contextmanager
    def _hook(output_dir, device_ids):
        import jax

        jax.devices()
        if device_ids:
            ids = (ctypes.c_int64 * len(device_ids))(*device_ids)
            rc = lib.axon_start_nrt_profile(ids, len(device_ids))
        else:
            rc = lib.axon_start_nrt_profile(None, 0)
        if rc != 0:
            raise RuntimeError(f"axon_start_nrt_profile rc={rc}")
        try:
            yield
        finally:
            n = lib.axon_stop_nrt_profile(str(output_dir).encode())
            print(f"profile: {n} file(s) written to {output_dir}")

    mod.set_axon_ntff_profile_hook(_hook)


_install_axon_hooks()

B, S, D, H, HD = 2, 4096, 256, 4, 64
NCORES = 8
QR = S // NCORES          # 512 query rows per core
KC = S // 128             # 32 key chunks of 128
SCALE = 1.0 / np.sqrt(HD)

_BUILT = {}


def _register_schra_op():
    """Register the fused (Src0*Src1*C0 + C1 clamped at 0 -> int16) DVE op."""
    from concourse import dve_ops
    from concourse.dve_spec import Spec, Src0, Src1, C0, C1, Zero, maxx, lower, _has_src1
    from concourse.dve_uop import DveOpSpec

    if "schra_op" in _BUILT:
        return _BUILT["schra_op"]
    name = "SCHRA_MULADD_ANT"
    for existing in dve_ops.OPS:
        if existing.name == name:  # re-import in the same process
            _BUILT["schra_op"] = existing
            return existing
    spec = Spec(
        body=maxx(Src0 * Src1 * C0 + C1, Zero),
        reference=lambda in0, in1, s0, s1, imm2: np.maximum(
            in0.astype(np.float32) * in1.astype(np.float32) * s0 + s1, 0.0
        ).astype(np.float32),
    )
    row = dve_ops._CUSTOM_DVE_ROW_BASE + len(dve_ops.OPS)
    shas = {}
    for ver in ("v3", "v4"):
        s = DveOpSpec(name=name, opcode=row, uops=lower(spec, ver=ver), rd1_en=_has_src1(spec))
        shas[ver] = s.sha(ver)
    op = dve_ops.DveOp(name, spec, subdim=False, uops_sha=shas)
    dve_ops.OPS.append(op)
    dve_ops.CUSTOM_DVE_SPECS[name] = spec
    dve_ops._SUB_OPCODE_FOR_NAME[name] = row
    _BUILT["schra_op"] = op
    return op


def build_bass():
    if "nc" in _BUILT:
        return _BUILT["nc"]

    import concourse.tile as tile
    from concourse import bacc, mybir

    f32, bf16, f16, i16 = (
        mybir.dt.float32, mybir.dt.bfloat16, mybir.dt.float16, mybir.dt.int16,
    )
    f32r = mybir.dt.float32r
    af = mybir.ActivationFunctionType
    alu = mybir.AluOpType
    schra = _register_schra_op()

    nc = bacc.Bacc("TRN2", target_bir_lowering=False, debug=False, num_devices=NCORES)

    xt_in = nc.dram_tensor("xt", [B, 2, 128, S], bf16, kind="ExternalInput")
    xqt_in = nc.dram_tensor("xqt", [B, 2, 128, QR], bf16, kind="ExternalInput")
    g_in = nc.dram_tensor("g", [128, KC, QR], f16, kind="ExternalInput")
    wq_in = nc.dram_tensor("wq", [2, 128, 256], bf16, kind="ExternalInput")
    wk_in = nc.dram_tensor("wk", [2, 128, 256], bf16, kind="ExternalInput")
    wv_in = nc.dram_tensor("wv", [2, 128, 256], bf16, kind="ExternalInput")
    wo_in = nc.dram_tensor("wo", [H, 64, 256], bf16, kind="ExternalInput")
    bias_in = nc.dram_tensor("bias", [1, 256], bf16, kind="ExternalInput")
    out_dram = nc.dram_tensor("out", [B, QR, 256], f32, kind="ExternalOutput")

    with tile.TileContext(nc) as tc, contextlib.ExitStack() as ctx:
        cp = ctx.enter_context(tc.tile_pool(name="const", bufs=1))
        # 3 slots x 2 banks for scores / KVQ staging / out-proj / rec-broadcast
        ps_big = ctx.enter_context(tc.tile_pool(name="ps_big", bufs=3, space="PSUM"))
        ps_ct = ctx.enter_context(tc.tile_pool(name="ps_ct", bufs=1, space="PSUM"))

        # ---- weights, DMA'd in dependency order: wk/wq gate the K/Q builds
        wk_sb = cp.tile([128, 2, 256], bf16, tag="wk")
        wq_sb = cp.tile([128, 2, 256], bf16, tag="wq")
        wv_sb = cp.tile([128, 2, 256], bf16, tag="wv")
        bias_sb = cp.tile([1, 256], bf16, tag="bias")
        ones_sb = cp.tile([1, 128], bf16, tag="ones")
        ones16_sb = cp.tile([1, 64], f16, tag="ones16")
        schrab_sb = cp.tile([128, 1], f32, tag="schrab")
        nc.gpsimd.memset(schrab_sb[:], SCHRA_B)
        nc.gpsimd.memset(ones_sb[:], 1.0)
        nc.gpsimd.memset(ones16_sb[:], 1.0)
        for ic in range(2):
            nc.sync.dma_start(wk_sb[:, ic, :], wk_in[ic])
            nc.sync.dma_start(wq_sb[:, ic, :], wq_in[ic])

        # xqt before xt so the Q projection (which gates the first scores)
        # isn't queued behind 4MB of x^T
        xqt_sb = cp.tile([128, B, 2, QR], bf16, tag="xqt")
        for b in range(B):
            for ic in range(2):
                nc.sync.dma_start(xqt_sb[:, b, ic, :], xqt_in[b, ic])

        xt_sb = [[cp.tile([128, S], bf16, tag=f"xt{b}{ic}", name=f"xt{b}{ic}") for ic in range(2)] for b in range(B)]
        # batch-0 x^T in column halves so the first K/V builds gate on 1MB;
        # the first 2 kc of G^T slot in right after so pair-0's elementwise
        # (t ~ +9us) never waits on G
        gt_sb = cp.tile([128, KC, QR], f16, tag="gt")
        for ic in range(2):
            nc.sync.dma_start(xt_sb[0][ic][:, 0:2048], xt_in[0, ic, :, 0:2048])
        nc.sync.dma_start(gt_sb[:, 0:2, :], g_in[:, 0:2, :])
        for ic in range(2):
            nc.sync.dma_start(wv_sb[:, ic, :], wv_in[ic])
        for ic in range(2):
            nc.sync.dma_start(xt_sb[0][ic][:, 2048:4096], xt_in[0, ic, :, 2048:4096])
        nc.sync.dma_start(gt_sb[:, 2:8, :], g_in[:, 2:8, :])
        for kp in range(1, 4):
            nc.sync.dma_start(gt_sb[:, kp * 8:(kp + 1) * 8, :], g_in[:, kp * 8:(kp + 1) * 8, :])
        for ic in range(2):
            nc.sync.dma_start(xt_sb[1][ic][:], xt_in[1, ic])
        wo_sb = []
        for h in range(H):
            t = cp.tile([64, 256], bf16, tag=f"wo{h}", name=f"wo{h}")
            nc.sync.dma_start(t[:], wo_in[h])
            wo_sb.append(t)
        nc.sync.dma_start(bias_sb[:], bias_in[:])

        qts = [[cp.tile([128, QR], bf16, tag=f"qt{b}{hp}", name=f"qt{b}{hp}") for hp in range(2)] for b in range(B)]

        vap = ctx.enter_context(tc.tile_pool(name="vap", bufs=2))
        stp = ctx.enter_context(tc.tile_pool(name="stp", bufs=4))
        ttp = ctx.enter_context(tc.tile_pool(name="ttp", bufs=4))
        pp = ctx.enter_context(tc.tile_pool(name="pp", bufs=5))
        otp = ctx.enter_context(tc.tile_pool(name="otp", bufs=2))
        rp = ctx.enter_context(tc.tile_pool(name="rp", bufs=2))

        ctf = [[None] * H for _ in range(B)]

        # va[keys, kc, 65h:65h+64] per batch, ones col at 65h+64 (denominators,
        # memset once up front; the per-sg copies skip those columns)
        vas = [vap.tile([128, KC, 260], f16, tag="va", name=f"va{b}") for b in range(B)]
        for b in range(B):
            nc.vector.memset(
                vas[b][:, :, :].rearrange("p k (h x) -> p k h x", h=4)[:, :, :, 64:65], 1.0
            )
        kts = [[cp.tile([128, S], bf16, tag=f"kt{b}{hp}", name=f"kt{b}{hp}") for hp in range(2)] for b in range(B)]

        def q_build(b, hp):
            aux = ps_big.tile([128, 2, 512], f32, tag="sc", name="auxq")
            for ic in range(2):
                nc.tensor.matmul(
                    aux[:, 0, :QR], wq_sb[:, ic, hp * 128:(hp + 1) * 128],
                    xqt_sb[:, b, ic, :], start=(ic == 0), stop=(ic == 1),
                )
            nc.vector.tensor_copy(qts[b][hp][:], aux[:, 0, :QR])

        def va_round(b, sg):
            va = vas[b]
            vps = ps_big.tile([128, 2, 512], f32, tag="sc", name="auxv")
            for j in range(2):
                kcj = sg * 2 + j
                for ic in range(2):
                    nc.tensor.matmul(
                        vps[:, j, 0:256],
                        xt_sb[b][ic][:, kcj * 128:(kcj + 1) * 128],
                        wv_sb[:, ic, :],
                        start=(ic == 0), stop=(ic == 1),
                    )
            nc.scalar.copy(
                va[:, sg * 2:(sg + 1) * 2, :].rearrange("p k (h x) -> p k h x", h=4)[:, :, :, 0:64],
                vps[:, 0:2, 0:256].rearrange("p j (h x) -> p j h x", h=4),
            )

        def kt_round(b, hp, sc4):
            kt = kts[b][hp]
            auxk = ps_big.tile([128, 2, 512], f32, tag="sc", name="auxk")
            for half in range(2):
                sc8 = sc4 * 2 + half
                for ic in range(2):
                    nc.tensor.matmul(
                        auxk[:, half, :], wk_sb[:, ic, hp * 128:(hp + 1) * 128],
                        xt_sb[b][ic][:, sc8 * 512:(sc8 + 1) * 512],
                        start=(ic == 0), stop=(ic == 1),
                    )
            nc.scalar.copy(
                kt[:, sc4 * 1024:(sc4 + 1) * 1024],
                auxk[:, 0:2, :].rearrange("p a b -> p (a b)"),
            )

        def out_proj(b):
            for qs in range(QR // 128):
                op = ps_big.tile([128, 2, 512], f32, tag="sc", name="auxo")
                for h in range(H):
                    nc.tensor.matmul(
                        op[:, 0, 0:256], ctf[b][h][:, qs * 128:(qs + 1) * 128],
                        wo_sb[h][:], start=(h == 0), stop=False,
                    )
                nc.tensor.matmul(op[:, 0, 0:256], ones_sb[0:1, :], bias_sb[0:1, :], start=False, stop=True)
                ot = otp.tile([128, 256], f32, tag="ot")
                nc.vector.tensor_copy(ot[:], op[:, 0, 0:256])
                nc.sync.dma_start(out_dram[b, qs * 128:(qs + 1) * 128, :], ot[:])

        # ---- minimal upfront builds; the rest is injected into pair loops
        for b in range(B):
            for hp in range(2):
                q_build(b, hp)
        for sc4 in range(4):
            kt_round(0, 0, sc4)
        for sg in range(6):
            va_round(0, sg)

        sched = {}

        def add(pidx, kc, fn):
            sched.setdefault((pidx, kc), []).append(fn)

        for i, sg in enumerate(range(6, 16)):          # va(0) tail
            add(0, 1 + 2 * i, lambda s=sg: va_round(0, s))
        for i, s4 in enumerate(range(4)):              # kt(0,1)
            add(0, 21 + 2 * i, lambda s=s4: kt_round(0, 1, s))
        for sg in range(16):                           # va(1)
            add(1, 1 + 2 * sg, lambda s=sg: va_round(1, s))
        for i, s4 in enumerate(range(4)):              # kt(1,0)
            add(1, 22 + 2 * i, lambda s=s4: kt_round(1, 0, s))
        for i, s4 in enumerate(range(4)):              # kt(1,1)
            add(2, 2 + 2 * i, lambda s=s4: kt_round(1, 1, s))
        add(2, 8, lambda: out_proj(0))

        for pidx, (b, hp) in enumerate([(0, 0), (0, 1), (1, 0), (1, 1)]):
            va = vas[b]
            qt = qts[b][hp]
            kt = kts[b][hp]

            # ---- main loop over key chunks ----
            ct0 = ps_ct.tile([65, QR], f32, tag="ct0")
            ct1 = ps_ct.tile([65, QR], f32, tag="ct1")
            pending_ctx = []

            def flush_ctx():
                for fn in pending_ctx:
                    fn()
                pending_ctx.clear()

            for kc in range(KC):
                scp = ps_big.tile([128, 2, QR], f32, tag="sc", name="scp")
                nc.tensor.matmul(
                    scp[:, 0, :], kt[0:64, kc * 128:(kc + 1) * 128], qt[0:64, :],
                    start=True, stop=True, tile_position=(0, 0),
                )
                nc.tensor.matmul(
                    scp[:, 1, :], kt[64:128, kc * 128:(kc + 1) * 128], qt[64:128, :],
                    start=True, stop=True, tile_position=(64, 0),
                )
                pt = pp.tile([128, 2, QR], f16, tag="pt")
                path = PATTERN32[kc]
                gtb = gt_sb[:, kc:kc + 1, :].broadcast_to([128, 2, QR])
                if path == "a":
                    # fused: i16 bits of fp16(e^(s*g)) straight from PSUM
                    nc.vector._custom_dve(
                        schra,
                        out=pt[:, :, :].bitcast(i16),
                        in0=scp[:, :, :],
                        in1=gtb,
                        s0=SCHRA_A, s1=SCHRA_B,
                    )
                else:
                    # staged per head so the drain->Pool-mul->relu chain is
                    # ~2.5us not ~5us: each head's ctx matmul waits only on
                    # its own half, keeping the in-order PE queue moving
                    sc16 = stp.tile([128, 2, QR], f16, tag="sc16")
                    tt = ttp.tile([128, 2, QR], f16, tag="tt")
                    for j in range(2):
                        nc.scalar.copy(sc16[:, j, :], scp[:, j, :])
                        nc.gpsimd.tensor_mul(tt[:, j, :], sc16[:, j, :], gt_sb[:, kc, :])
                        # Schraudolph tail on ACT: relu(t*A+B) -> i16
                        nc.scalar.activation(
                            pt[:, j, :].bitcast(i16), tt[:, j, :],
                            af.Relu, scale=SCHRA_A, bias=schrab_sb[:, :],
                        )
                # emit this step's ctx matmuls one step LATE (after the next
                # step's scores/elementwise): the in-order PE queue then has
                # ~1 extra step of work before it blocks on pt_kc, covering
                # the d-path's ~2.5us drain->Pool->relu chain latency
                h0 = 65 * (2 * hp)
                h1 = 65 * (2 * hp + 1)

                def emit_ctx(kc=kc, pt=pt):
                    nc.tensor.matmul(
                        ct0[:, :], va[:, kc, h0:h0 + 65], pt[:, 0, :],
                        start=(kc == 0), stop=(kc == KC - 1),
                    )
                    nc.tensor.matmul(
                        ct1[:, :], va[:, kc, h1:h1 + 65], pt[:, 1, :],
                        start=(kc == 0), stop=(kc == KC - 1),
                    )

                prev, pending_ctx[:] = pending_ctx[:], [emit_ctx]
                for fn in prev:
                    fn()
                for fn in sched.get((pidx, kc), ()):
                    fn()
            flush_ctx()

            # ---- stash unnormalized ctx^T; per-pair 64/denom dance ----
            # high_priority: the ct0/ct1 release (and the next pair's ctx
            # matmuls behind it) must not queue after leftover ACT/DVE work
            with tc.high_priority():
                c0 = cp.tile([64, QR], bf16, tag=f"ctf{b}_{2 * hp}", name=f"ctf{b}_{2 * hp}")
                c1 = cp.tile([64, QR], bf16, tag=f"ctf{b}_{2 * hp + 1}", name=f"ctf{b}_{2 * hp + 1}")
                nc.scalar.copy(c0[:], ct0[0:64, :])
                nc.scalar.copy(c1[:], ct1[0:64, :])
                den = rp.tile([1, 2, QR], f32, tag="den")
                nc.vector.tensor_copy(den[0:1, 0, :], ct0[64:65, :])
                nc.vector.tensor_copy(den[0:1, 1, :], ct1[64:65, :])
                rec = rp.tile([1, 2, QR], f32, tag="rec")
                nc.vector.reciprocal_approx_fast(
                    rec[0:1, :, :].rearrange("p a b -> p (a b)"),
                    den[0:1, :, :].rearrange("p a b -> p (a b)"),
                )
                # rec16 = f16(64/den): the x64 dodges f16 subnormals (1/den can
                # be ~6e-5); Wo is pre-divided by 64 on the host to compensate.
                rec16 = rp.tile([1, 2, QR], f16, tag="rec16")
                nc.scalar.mul(rec16[0:1, :, :].rearrange("p a b -> p (a b)"),
                              rec[0:1, :, :].rearrange("p a b -> p (a b)"), 64.0)
                # broadcast across 64 partitions with a K=1 f16 matmul (~0.2us)
                bcp = ps_big.tile([128, 2, 512], f32, tag="sc", name="auxb")
                for j, cj in ((0, c0), (1, c1)):
                    nc.tensor.matmul(
                        bcp[0:64, j, :], ones16_sb[0:1, :], rec16[0:1, j, :],
                        start=True, stop=True,
                    )
                    nc.vector.tensor_mul(cj[:], cj[:], bcp[0:64, j, :])
            ctf[b][2 * hp] = c0
            ctf[b][2 * hp + 1] = c1

        out_proj(1)

    nc.compile()
    _BUILT["nc"] = nc
    return nc


def host_inputs(x, G, Wq, Wk, Wv, Wo, bo, b_extra):
    """Build the per-core input maps (layout prep + query-row sharding)."""
    import ml_dtypes

    f = np.float32
    bf = ml_dtypes.bfloat16
    x = np.asarray(x, f)
    G = np.asarray(G, f)
    xt = np.ascontiguousarray(x.transpose(0, 2, 1)).reshape(B, 2, 128, S).astype(bf)
    wq = np.ascontiguousarray(np.asarray(Wq, f).T * SCALE).reshape(2, 128, 256).astype(bf)
    wk = np.ascontiguousarray(np.asarray(Wk, f).T).reshape(2, 128, 256).astype(bf)
    wv = np.ascontiguousarray(np.asarray(Wv, f).T).reshape(2, 128, 256).astype(bf)
    # Wo/64 compensates the 64/den normalization scale (see kernel pair-end)
    wo = np.ascontiguousarray(np.asarray(Wo, f).T / 64.0).reshape(H, 64, 256).astype(bf)
    bias = (np.asarray(bo, f) + np.asarray(b_extra, f)).reshape(1, 256).astype(bf)

    shared = {"xt": xt, "wq": wq, "wk": wk, "wv": wv, "wo": wo, "bias": bias}
    in_maps = []
    for c in range(NCORES):
        q0 = c * QR
        m = dict(shared)
        # host-side transpose to gt[p, kc, q] = G[q0+q, kc*128+p]
        gc = G[q0:q0 + QR, :].T.reshape(KC, 128, QR)
        m["g"] = np.ascontiguousarray(gc.transpose(1, 0, 2)).astype(np.float16)
        m["xqt"] = np.ascontiguousarray(xt[:, :, :, q0:q0 + QR])
        in_maps.append(m)
    return in_maps


def run(in_maps, trace=False):
    from concourse.bass_utils import run_bass_kernel_spmd

    nc = build_bass()
    return run_bass_kernel_spmd(nc, in_maps, core_ids=list(range(NCORES)), trace=trace)


def kernel(x, G, Wq, Wk, Wv, Wo, bo, b_extra):
    in_maps = host_inputs(x, G, Wq, Wk, Wv, Wo, bo, b_extra)
    res = run(in_maps, trace=False)
    out = np.concatenate([res.results[c]["out"] for c in range(NCORES)], axis=1)
    return out.astype(np.float32)
